# revision 17
# baseline (speedup 1.0000x reference)
"""Multi-head attention (B=4, S=2048, D=1024, H=16, causal) on 8 trn2 cores.

Sharding: core c -> batch b = c//2, heads h in [8*(c%2), 8*(c%2)+8).
Each core computes its 8 heads' probs ([8, S, S], lower triangle only --
output buffers are pre-zeroed) and a partial output projection
(ctx_heads @ Wo.T rows); host sums the two partials per batch (+bo).

Orientation trick: scores are computed twice, in both layouts:
  - S^T[t, s] feeds exp -> ctx^T accumulation (lhsT = [v | ones] gives
    row-sums for free in PSUM row 64); ctx^T is exactly the lhsT the
    output projection needs.
  - S[s, t] feeds exp(+accum row sums) -> normalized probs written to HBM.
All matmuls run as float32r (FP22) which is full-rate for N>=256.
Causal masking = tiny bf16 matmuls accumulating -1e9 above the diagonal.
"""

import os

import numpy as np

PHASES = os.environ.get("KERNEL_PHASES", "1ACB")

S = 2048
D = 1024
HPC = 8          # heads per core
DK = 64
DH = HPC * DK    # 512, per-core head width
NST = S // 128   # 16 s-tiles
NSC = S // 512   # 4 s-chunks
NKC = D // 128   # 8 k-chunks
NEG = -1.0e9

_CACHE = {}


def _build():
    import concourse.bass as bass
    import concourse.tile as tile
    from concourse import bacc, mybir

    f32 = mybir.dt.float32
    f32r = mybir.dt.float32r
    bf16 = mybir.dt.bfloat16
    Exp = mybir.ActivationFunctionType.Exp

    nc = bacc.Bacc("TRN2", target_bir_lowering=False, debug=False,
                   enable_asserts=False, num_devices=8)

    QT = nc.dram_tensor("QT", [D, S], f32r, kind="ExternalInput").ap()
    KT = nc.dram_tensor("KT", [D, S], f32r, kind="ExternalInput").ap()
    VT = nc.dram_tensor("VT", [D, S], f32r, kind="ExternalInput").ap()
    WQT = nc.dram_tensor("WQT", [D, DH], f32r, kind="ExternalInput").ap()
    WKT = nc.dram_tensor("WKT", [D, DH], f32r, kind="ExternalInput").ap()
    WVT = nc.dram_tensor("WVT", [D, DH], f32r, kind="ExternalInput").ap()
    WOT = nc.dram_tensor("WOT", [DH, D], f32r, kind="ExternalInput").ap()
    BQ = nc.dram_tensor("BQ", [128, 4], f32, kind="ExternalInput").ap()
    BK = nc.dram_tensor("BK", [128, 4], f32, kind="ExternalInput").ap()
    BVB = nc.dram_tensor("BVB", [128, DH], f32, kind="ExternalInput").ap()
    PROBS = nc.dram_tensor("PROBS", [HPC, S, S], f32, kind="ExternalOutput").ap()
    OUTP = nc.dram_tensor("OUTP", [S, D], f32, kind="ExternalOutput").ap()

    with tile.TileContext(nc) as tc:
        _trace(tc, nc, bass, mybir, f32, f32r, bf16, Exp,
               QT, KT, VT, WQT, WKT, WVT, WOT, BQ, BK, BVB, PROBS, OUTP)
    nc.compile()
    return nc


def _trace(tc, nc, bass, mybir, f32, f32r, bf16, Exp,
           QT, KT, VT, WQT, WKT, WVT, WOT, BQ, BK, BVB, PROBS, OUTP):
    Alu = mybir.AluOpType

    # ---------- persistent SBUF ----------
    persist = tc.alloc_tile_pool(name="persist", bufs=1)
    qT_sb = persist.tile([128, 4, S], f32r)     # d' = m*128+p, per-core q^T / 8
    kT_sb = persist.tile([128, 4, S], f32r)
    v_sb = persist.tile([128, NST, HPC, DK + 1], f32r)  # [t%128, ti, h, dk|1]
    ctxT_sb = persist.tile([128, 4, S], f32r)   # normalized ctx^T

    consts = tc.alloc_tile_pool(name="consts", bufs=1)
    bq_sb = consts.tile([128, 4], f32)
    bk_sb = consts.tile([128, 4], f32)
    bvb_sb = consts.tile([128, DH], f32)
    negT1_bf = consts.tile([128, 128], bf16)   # -1e9 strictly above diagonal
    i_bf = consts.tile([128, 128], bf16)       # identity
    nc.sync.dma_start(out=bq_sb, in_=BQ)
    nc.sync.dma_start(out=bk_sb, in_=BK)
    nc.sync.dma_start(out=bvb_sb, in_=BVB)

    with tc.tile_pool(name="maskbuild", bufs=1) as mb_pool:
        negT1_f = mb_pool.tile([128, 128], f32)
        nc.gpsimd.memset(negT1_f, 0.0)
        # out[x, y] = (x - y) >= 0 ? in_ : NEG  -> NEG strictly above diag
        nc.gpsimd.affine_select(out=negT1_f, in_=negT1_f, compare_op=Alu.is_ge,
                                fill=NEG, base=0, pattern=[[-1, 128]],
                                channel_multiplier=1)
        nc.vector.tensor_copy(negT1_bf, negT1_f)
        i_f = mb_pool.tile([128, 128], f32)
        from concourse.masks import make_identity
        make_identity(nc, i_f)
        nc.vector.tensor_copy(i_bf, i_f)

    nc.vector.memset(v_sb[:, :, :, DK:DK + 1].bitcast(f32), 1.0)

    def q_h(h, lo, n):
        return qT_sb[64 * (h % 2):64 * (h % 2) + 64, h // 2, lo:lo + n]

    def k_h(h, lo, n):
        return kT_sb[64 * (h % 2):64 * (h % 2) + 64, h // 2, lo:lo + n]

    # Attention-phase SBUF pools allocated BEFORE the (released) projection
    # pools so the projection pools sit on top of the stack and their
    # release does not gate attention tiles.
    eB_pool = tc.alloc_tile_pool(name="pB_e", bufs=2)
    n_pool = tc.alloc_tile_pool(name="pA_n", bufs=2)
    s_pool = tc.alloc_tile_pool(name="pB_s", bufs=6)
    # Single shared PSUM rotation for ALL phases: "sc" [128,1024] (2 banks)
    # x2 bufs + ctx halves [65,1024] (2 banks) x2 bufs = 8 banks.
    sp_pool = tc.alloc_tile_pool(name="psc", bufs=2, space="PSUM")
    ps_ctx_pool = tc.alloc_tile_pool(name="pctx", bufs=2, space="PSUM")

    # ---------- phase 1: q/k/v projections (q, k first; v streams last) ----
    with tc.tile_pool(name="p1w", bufs=1) as wpool, \
         tc.tile_pool(name="p1in", bufs=2) as inpool:

        def proj_pass(W_dram, X_dram, kind):
            w_sb = wpool.tile([128, NKC, DH], f32r, tag="w")
            wr = W_dram.rearrange("(k p) m -> p k m", p=128)
            for k in range(NKC):
                nc.sync.dma_start(out=w_sb[:, k, :], in_=wr[:, k, :])
            xr = X_dram.rearrange("(k p) s -> p k s", p=128)
            for c in range(NSC):
                x_in = inpool.tile([128, NKC, 512], f32r, tag="xin")
                for k in range(NKC):
                    nc.sync.dma_start(out=x_in[:, k, :],
                                      in_=xr[:, k, 512 * c:512 * (c + 1)])
                for m in range(4):
                    ps = ps_ctx_pool.tile([128, 512], f32, tag="ctx")
                    for k in range(NKC):
                        if kind == "v":
                            lhsT = x_in[:, k, 128 * m:128 * (m + 1)]
                            rhs = w_sb[:, k, :]
                        else:
                            lhsT = w_sb[:, k, 128 * m:128 * (m + 1)]
                            rhs = x_in[:, k, :]
                        nc.tensor.matmul(ps, lhsT, rhs,
                                         start=(k == 0), stop=(k == NKC - 1))
                    if kind == "q":
                        nc.vector.tensor_scalar_add(
                            qT_sb[:, m, 512 * c:512 * (c + 1)], ps, bq_sb[:, m:m + 1])
                    elif kind == "k":
                        nc.vector.tensor_scalar_add(
                            kT_sb[:, m, 512 * c:512 * (c + 1)], ps, bk_sb[:, m:m + 1])
                    else:
                        ti = 4 * c + m
                        nc.vector.tensor_add(
                            v_sb[:, ti, :, 0:DK],
                            ps.rearrange("p (h d) -> p h d", h=HPC),
                            bvb_sb.rearrange("p (h d) -> p h d", h=HPC))

        proj_pass(WQT, QT, "q")
        proj_pass(WKT, KT, "k")
        proj_pass(WVT, VT, "v")

    # allocated after the projection pools so it reuses their freed space
    e_pool = tc.alloc_tile_pool(name="pA_e", bufs=3)

    # ---------- phases A+B interleaved per head ----------
    wo_pool = tc.alloc_tile_pool(name="pCw", bufs=1)
    o_pool = tc.alloc_tile_pool(name="pCo", bufs=2)
    wo_sb = wo_pool.tile([128, 4, D], f32r)
    if "C" in PHASES:
        nc.sync.dma_start(out=wo_sb, in_=WOT.rearrange("(m p) n -> p m n", p=128))

    for h in range(HPC):
        hb = 64 * (h % 2)
        if "A" in PHASES:
            ctx01 = [ps_ctx_pool.tile([DK + 1, 1024], f32, tag="ctx",
                                      name=f"ctx{h}_{p}") for p in range(2)]

        def a_tile(p, ti):
            ctx_ps = ctx01[p]
            c0, rr = ti // 4, ti % 4
            sc = sp_pool.tile([128, 1024], f32, tag="sc", name=f"scA{h}_{p}_{ti}")
            for c in range(max(2 * p, c0), 2 * p + 2):
                off = (c - 2 * p) * 512
                sl = min(128 * rr, 256) if c == c0 else 0
                nc.tensor.matmul(
                    sc[:, off + sl:off + 512],
                    k_h(h, 128 * ti, 128),
                    qT_sb[hb:hb + 64, h // 2, 512 * c + sl:512 * (c + 1)],
                    start=True, stop=(c != c0))
                if c == c0:
                    dcol = off + 128 * rr
                    nc.tensor.matmul(sc[:, dcol:dcol + 128],
                                     negT1_bf, i_bf, start=False, stop=True)
            exp_lo = (c0 - 2 * p) * 512 + 128 * rr if c0 >= 2 * p else 0
            e = e_pool.tile([128, 1024], f32r, tag="e", name=f"eA{h}_{p}_{ti}")
            nc.scalar.activation(e[:, exp_lo:1024], sc[:, exp_lo:1024], Exp)
            for c in range(max(2 * p, c0), 2 * p + 2):
                off = (c - 2 * p) * 512
                dlo = 128 * rr if c == c0 else 0
                nc.tensor.matmul(
                    ctx_ps[:, off + dlo:off + 512],
                    v_sb[:, ti, h, :],
                    e[:, off + dlo:off + 512],
                    start=(ti == 0), stop=(ti == 4 * c + 3))

        def a_norm(p):
            ctx_ps = ctx01[p]
            rrow = n_pool.tile([1, 1024], f32, tag="nrm", name=f"rr{h}_{p}")
            nc.vector.reciprocal(rrow, ctx_ps[DK:DK + 1, :])
            rbc = n_pool.tile([64, 1024], f32, tag="nrm", name=f"rb{h}_{p}")
            nc.gpsimd.partition_broadcast(rbc, rrow)
            nc.vector.tensor_mul(
                ctxT_sb[hb:hb + 64, h // 2, 1024 * p:1024 * (p + 1)],
                ctx_ps[0:DK, :], rbc)

        def b_row(si):
            L = (si + 1) * 128
            nh = (L + 1023) // 1024
            e = eB_pool.tile([128, 2048], f32, tag="eb", name=f"eB{h}_{si}")
            parts = s_pool.tile([128, 2], f32, tag="pp", name=f"pp{h}_{si}")
            for hi in range(nh):
                hl = min(1024, L - 1024 * hi)
                rp = sp_pool.tile([128, 1024], f32, tag="sc", name=f"scB{h}_{si}_{hi}")
                for c in range(2 * hi, 2 * hi + (hl + 511) // 512):
                    off = (c - 2 * hi) * 512
                    n = min(512, L - 512 * c)
                    n_mm = 512 if n == 512 else max(n, 256)
                    last = (c == si // 4)
                    nc.tensor.matmul(rp[:, off:off + n_mm],
                                     q_h(h, 128 * si, 128),
                                     k_h(h, 512 * c, n_mm),
                                     start=True, stop=not last)
                    if last:
                        dcol = off + 128 * si - 512 * c
                        nc.tensor.matmul(rp[:, dcol:dcol + 128],
                                         i_bf, negT1_bf, start=False, stop=True)
                nc.scalar.activation(e[:, 1024 * hi:1024 * hi + hl],
                                     rp[:, 0:hl], Exp,
                                     accum_out=parts[:, hi:hi + 1])
            ssum = s_pool.tile([128, 1], f32, tag="ss", name=f"ss{h}_{si}")
            nc.vector.reduce_sum(ssum, parts[:, 0:nh], axis=mybir.AxisListType.X)
            rcp = s_pool.tile([128, 1], f32, tag="rc", name=f"rc{h}_{si}")
            nc.vector.reciprocal(rcp, ssum)
            nc.vector.tensor_scalar_mul(e[:, 0:L], e[:, 0:L], rcp)
            nc.sync.dma_start(out=PROBS[h, 128 * si:128 * (si + 1), 0:L],
                              in_=e[:, 0:L])

        def c_group(j):
            st, nn = j // 2, j % 2
            po = sp_pool.tile([128, 512], f32, tag="sc", name=f"po{j}")
            for m in range(4):
                nc.tensor.matmul(po,
                                 ctxT_sb[:, m, 128 * st:128 * (st + 1)],
                                 wo_sb[:, m, 512 * nn:512 * (nn + 1)],
                                 start=(m == 0), stop=(m == 3))
            ob = o_pool.tile([128, 512], f32, tag="ob", name=f"ob{j}")
            nc.vector.tensor_copy(ob, po)
            nc.sync.dma_start(
                out=OUTP[128 * st:128 * (st + 1), 512 * nn:512 * (nn + 1)],
                in_=ob)

        if h < HPC - 1:
            for i in range(NST):
                if "A" in PHASES:
                    if i < 8:
                        a_tile(0, i)
                    a_tile(1, i)
                    if i == 7:
                        a_norm(0)
                if "B" in PHASES:
                    b_row(i)
            if "A" in PHASES:
                a_norm(1)
        else:
            # last head: finish A first, then overlap B with phase C
            if "A" in PHASES:
                for i in range(NST):
                    if i < 8:
                        a_tile(0, i)
                    a_tile(1, i)
                    if i == 7:
                        a_norm(0)
                a_norm(1)
            for i in range(NST):
                if "B" in PHASES:
                    b_row(i)
                if "C" in PHASES:
                    c_group(2 * i)
                    c_group(2 * i + 1)

    if "A" not in PHASES:
        nc.vector.memset(ctxT_sb.bitcast(f32), 0.0)

    o_pool.release()
    wo_pool.release()
    e_pool.release()
    ps_ctx_pool.release()
    sp_pool.release()
    s_pool.release()
    n_pool.release()
    eB_pool.release()
    consts.release()
    persist.release()


def _get_nc():
    if "nc" not in _CACHE:
        _CACHE["nc"] = _build()
    return _CACHE["nc"]


def _shard_inputs(Q, K, V, Wq, bq, Wk, bk, Wv, bv, Wo):
    Q = np.asarray(Q, np.float32)
    K = np.asarray(K, np.float32)
    V = np.asarray(V, np.float32)
    in_maps = []
    for core in range(8):
        b = core // 2
        hs = core % 2
        sl = slice(DH * hs, DH * (hs + 1))
        in_maps.append({
            "QT": np.ascontiguousarray(Q[b].T),
            "KT": np.ascontiguousarray(K[b].T),
            "VT": np.ascontiguousarray(V[b].T),
            "WQT": np.ascontiguousarray((np.asarray(Wq, np.float32)[sl] / 8.0).T),
            "WKT": np.ascontiguousarray(np.asarray(Wk, np.float32)[sl].T),
            "WVT": np.ascontiguousarray(np.asarray(Wv, np.float32)[sl].T),
            "WOT": np.ascontiguousarray(np.asarray(Wo, np.float32)[:, sl].T),
            "BQ": np.ascontiguousarray(
                (np.asarray(bq, np.float32)[sl] / 8.0).reshape(4, 128).T),
            "BK": np.ascontiguousarray(
                np.asarray(bk, np.float32)[sl].reshape(4, 128).T),
            "BVB": np.ascontiguousarray(
                np.broadcast_to(np.asarray(bv, np.float32)[sl], (128, DH))),
        })
    return in_maps


def run_sharded(in_maps, trace=False):
    from concourse import bass_utils
    nc = _get_nc()
    return bass_utils.run_bass_kernel_spmd(nc, in_maps, core_ids=list(range(8)),
                                           trace=trace)


def kernel(Q, K, V, mask, Wq, bq, Wk, bk, Wv, bv, Wo, bo):
    in_maps = _shard_inputs(Q, K, V, Wq, bq, Wk, bk, Wv, bv, Wo)
    res = run_sharded(in_maps).results
    B = 4
    H = 16
    bo = np.asarray(bo, np.float32)
    out = np.empty((B, S, D), np.float32)
    probs = np.empty((B, H, S, S), np.float32)
    for b in range(B):
        out[b] = res[2 * b]["OUTP"] + res[2 * b + 1]["OUTP"] + bo
        probs[b, 0:HPC] = res[2 * b]["PROBS"]
        probs[b, HPC:H] = res[2 * b + 1]["PROBS"]
    return out, probs
